# revision 4
# baseline (speedup 1.0000x reference)
"""Distributed MemoryCenters read kernel for 8 Trainium2 NeuronCores.

Strategy (sharded-kNN per the distributed top-k pattern):
  - Shard the center table K along n_centers across the 8 cores
    (12500 centers each). Queries are replicated.
  - Each core computes sim = q @ K_shard^T on the PE (float32r = full-rate
    fp32), and extracts top-8 candidates (values + indices) per 1536-wide
    chunk of its shard with the DVE max8 / max_index instructions.
  - The host merges the 8 * 72 = 576 candidates per query, takes the global
    top-32 by RBF weight (exactly reproducing the reference's ordering and
    tie-breaking), and performs the cheap O(k) softmax / gather reduction.

Exactness: top-8 per 1536-chunk is an exact cover of the global top-32 as
long as no chunk holds more than 8 of the top-32 (the actual maximum on this
data distribution is 5; the merge would detect a violation via the
saturation check in test.py).
"""

import numpy as np

SIGMA_READ = 0.5
EPS = 1e-8

B, T, D = 2, 512, 128
N, DV, DE = 100000, 256, 4
NCORES = 8
NSHARD = N // NCORES  # 12500
Q = B * T  # 1024
QTILES = Q // 128  # 8

CHUNK = 1536
# (offset, width) chunks covering the 12500-wide shard
CHUNKS = [(i * CHUNK, min(CHUNK, NSHARD - i * CHUNK)) for i in range((NSHARD + CHUNK - 1) // CHUNK)]
NCHUNK = len(CHUNKS)  # 9
CAND = NCHUNK * 8  # 72 candidates per (query, core)

_CACHE = {}


def _build_bass():
    """Build + compile the per-core Bass program (identical on all cores)."""
    from contextlib import ExitStack

    import concourse.bacc as bacc
    import concourse.mybir as mybir
    import concourse.tile as tile

    f32 = mybir.dt.float32
    f32r = mybir.dt.float32r
    u32 = mybir.dt.uint32

    nc = bacc.Bacc("TRN2", target_bir_lowering=False, debug=False,
                   enable_asserts=False, num_devices=NCORES)

    qT = nc.dram_tensor("qT", [128, Q], f32, kind="ExternalInput").ap()
    kt = nc.dram_tensor("kt", [128, NSHARD], f32, kind="ExternalInput").ap()
    outv = nc.dram_tensor("outv", [QTILES, 128, CAND], f32, kind="ExternalOutput").ap()
    outi = nc.dram_tensor("outi", [QTILES, 128, CAND], u32, kind="ExternalOutput").ap()

    with tile.TileContext(nc) as tc, ExitStack() as ctx:
        kt_pool = ctx.enter_context(tc.tile_pool(name="kt", bufs=1))
        q_pool = ctx.enter_context(tc.tile_pool(name="q", bufs=1))
        psum_pool = ctx.enter_context(tc.tile_pool(name="ps", bufs=2, space="PSUM"))
        chunk_pool = ctx.enter_context(tc.tile_pool(name="ch", bufs=4))
        out_pool = ctx.enter_context(tc.tile_pool(name="out", bufs=2))

        kt_sb = kt_pool.tile([128, NSHARD], f32)
        # split the 6.4MB K^T load across several DMAs for queue parallelism
        ndma = 10
        step = (NSHARD + ndma - 1) // ndma
        for i in range(ndma):
            s, e2 = i * step, min(NSHARD, (i + 1) * step)
            nc.sync.dma_start(out=kt_sb[:, s:e2], in_=kt[:, s:e2])
        q_sb = q_pool.tile([128, Q], f32)
        nc.sync.dma_start(out=q_sb[:], in_=qT[:])

        for t in range(QTILES):
            ov = out_pool.tile([128, CAND], f32, tag="ov")
            oi = out_pool.tile([128, CAND], u32, tag="oi")
            lhs = q_sb[:, t * 128:(t + 1) * 128]
            for ci, (off, w) in enumerate(CHUNKS):
                ps = psum_pool.tile([128, CHUNK], f32, tag="ps")
                for j in range(0, w, 512):
                    ww = min(512, w - j)
                    nc.tensor.matmul(
                        ps[:, j:j + ww],
                        lhsT=lhs,
                        rhs=kt_sb[:, off + j: off + j + ww],
                        start=True, stop=True,
                    )
                sb = chunk_pool.tile([128, CHUNK], f32, tag="ch")
                nc.scalar.activation(sb[:, :w], ps[:, :w],
                                     mybir.ActivationFunctionType.Copy)
                nc.vector.max(ov[:, ci * 8:(ci + 1) * 8], sb[:, :w])
                nc.vector.max_index(oi[:, ci * 8:(ci + 1) * 8],
                                    ov[:, ci * 8:(ci + 1) * 8], sb[:, :w])
            nc.sync.dma_start(out=outv[t], in_=ov[:])
            nc.sync.dma_start(out=outi[t], in_=oi[:])

    nc.compile()
    return nc


def _get_compiled():
    if "nc" not in _CACHE:
        _CACHE["nc"] = _build_bass()
    return _CACHE["nc"]


def kernel(queries, K, V, h, e, top_k):
    top_k = int(top_k)
    assert top_k == 32, top_k
    queries = np.asarray(queries, dtype=np.float32)
    K = np.asarray(K, dtype=np.float32)
    V = np.asarray(V, dtype=np.float32)
    h = np.asarray(h, dtype=np.float32)
    e = np.asarray(e, dtype=np.float32)

    # ---- shard + run on 8 NeuronCores ----
    qT = np.ascontiguousarray(queries.reshape(Q, D).T)  # [128, 1024]
    in_maps = []
    for c in range(NCORES):
        sh = np.ascontiguousarray(K[c * NSHARD:(c + 1) * NSHARD].T)  # [128, 12500]
        in_maps.append({"qT": qT, "kt": sh})

    from concourse.bass_utils import run_bass_kernel_spmd

    nc = _get_compiled()
    res = run_bass_kernel_spmd(nc, in_maps, list(range(NCORES))).results

    # ---- unshard: merge the 8*72 candidates per query ----
    vals = np.stack([res[c]["outv"] for c in range(NCORES)])  # [8, QTILES, 128, CAND]
    idxs = np.stack([res[c]["outi"] for c in range(NCORES)]).astype(np.int64)
    cand_off = np.repeat(np.array([off for off, _ in CHUNKS], dtype=np.int64), 8)
    gidx = idxs + cand_off[None, None, None, :]
    gidx += (np.arange(NCORES, dtype=np.int64) * NSHARD)[:, None, None, None]

    v = vals.transpose(1, 2, 0, 3).reshape(Q, NCORES * CAND)
    gi = gidx.transpose(1, 2, 0, 3).reshape(Q, NCORES * CAND)

    # RBF weights, computed exactly as the reference does (f32 throughout)
    dist_sq = np.float32(2.0) - np.float32(2.0) * v
    rbf = np.exp(-dist_sq / np.float32(2.0 * SIGMA_READ ** 2)).astype(np.float32)

    # global top-32 by rbf, ties broken by lower center index (lax.top_k order)
    order = np.lexsort((gi, -rbf.astype(np.float64)), axis=1)[:, :top_k]
    topk_idx = np.take_along_axis(gi, order, axis=1)  # [Q, 32]
    topk_w = np.take_along_axis(rbf, order, axis=1)  # [Q, 32]

    # ---- final O(k) reduction, replicating the reference numerics ----
    h_topk = h[topk_idx]
    log_w = np.log(topk_w + np.float32(EPS)) + np.log(h_topk + np.float32(EPS))
    m = log_w.max(axis=-1, keepdims=True)
    ew = np.exp(log_w - m)
    weights = (ew / ew.sum(axis=-1, keepdims=True)).astype(np.float32)

    V_sel = V[topk_idx]  # [Q, 32, DV]
    e_sel = e[topk_idx]  # [Q, 32, DE]
    r_V = np.einsum('qk,qkv->qv', weights, V_sel).astype(np.float32)
    r_E = np.einsum('qk,qke->qe', weights, e_sel).astype(np.float32)

    return (
        r_V.reshape(B, T, DV),
        r_E.reshape(B, T, DE),
        weights.reshape(B, T, top_k),
        topk_idx.reshape(B, T, top_k).astype(np.int32),
    )


# revision 6
# speedup vs baseline: 1.0221x; 1.0221x over previous
"""Distributed MemoryCenters read kernel for 8 Trainium2 NeuronCores.

Strategy (sharded-kNN per the distributed top-k pattern):
  - Shard the center table K along n_centers across the 8 cores
    (12500 centers each). Queries are replicated.
  - Each core computes sim = q @ K_shard^T on the PE. To get fp32-grade
    precision at full PE rate, operands are split hi/lo (hi = fp16 with
    subnormals flushed on host, lo = bf16 residual) and accumulated as
    qh*Kh + qh*Kl + ql*Kh in one PSUM group (measured max err 1.9e-7,
    same as fp32).
  - Each core extracts top-8 candidates (values + in-chunk indices) per
    2048-wide chunk of its shard with the DVE max8 / find_index8 ops.
  - The host merges the 8 * 56 = 448 candidates per query, takes the
    global top-32 by RBF weight (reproducing the reference's ordering and
    tie-breaking), and performs the cheap O(k) softmax / gather reduction.

Exactness: top-8 per 2048-chunk covers the global top-32 as long as no
chunk holds more than 8 of the top-32 (actual maximum on this data is 5;
test.py's saturation check proves this per-run).
"""

import numpy as np

SIGMA_READ = 0.5
EPS = 1e-8

B, T, D = 2, 512, 128
N, DV, DE = 100000, 256, 4
NCORES = 8
NSHARD = N // NCORES  # 12500
Q = B * T  # 1024
QTILES = Q // 128  # 8

CHUNK = 2048
# (offset, width) chunks covering the 12500-wide shard
CHUNKS = [(i * CHUNK, min(CHUNK, NSHARD - i * CHUNK)) for i in range((NSHARD + CHUNK - 1) // CHUNK)]
NCHUNK = len(CHUNKS)  # 7
CAND = NCHUNK * 8  # 56 candidates per (query, core)

F16_MIN_NORMAL = 6.2e-05  # flush-to-zero threshold for the hi fp16 part

_CACHE = {}


def _build_bass():
    """Build + compile the per-core Bass program (identical on all cores)."""
    from contextlib import ExitStack

    import concourse.bacc as bacc
    import concourse.mybir as mybir
    import concourse.tile as tile

    f32 = mybir.dt.float32
    f16 = mybir.dt.float16
    bf16 = mybir.dt.bfloat16
    u32 = mybir.dt.uint32

    nc = bacc.Bacc("TRN2", target_bir_lowering=False, debug=False,
                   enable_asserts=False, num_devices=NCORES)

    qh = nc.dram_tensor("qh", [128, Q], f16, kind="ExternalInput").ap()
    ql = nc.dram_tensor("ql", [128, Q], bf16, kind="ExternalInput").ap()
    kh = nc.dram_tensor("kh", [128, NSHARD], f16, kind="ExternalInput").ap()
    kl = nc.dram_tensor("kl", [128, NSHARD], bf16, kind="ExternalInput").ap()
    outv = nc.dram_tensor("outv", [QTILES, 128, CAND], f32, kind="ExternalOutput").ap()
    outi = nc.dram_tensor("outi", [QTILES, 128, CAND], u32, kind="ExternalOutput").ap()

    with tile.TileContext(nc) as tc, ExitStack() as ctx:
        k_pool = ctx.enter_context(tc.tile_pool(name="k", bufs=1))
        q_pool = ctx.enter_context(tc.tile_pool(name="q", bufs=1))
        psum_pool = ctx.enter_context(tc.tile_pool(name="ps", bufs=2, space="PSUM"))
        chunk_pool = ctx.enter_context(tc.tile_pool(name="ch", bufs=4))
        out_pool = ctx.enter_context(tc.tile_pool(name="out", bufs=2))

        kh_sb = k_pool.tile([128, NSHARD], f16, tag="kh")
        kl_sb = k_pool.tile([128, NSHARD], bf16, tag="kl")
        ndma = 5
        step = (NSHARD + ndma - 1) // ndma
        for i in range(ndma):
            s, e2 = i * step, min(NSHARD, (i + 1) * step)
            nc.sync.dma_start(out=kh_sb[:, s:e2], in_=kh[:, s:e2])
            nc.sync.dma_start(out=kl_sb[:, s:e2], in_=kl[:, s:e2])
        qh_sb = q_pool.tile([128, Q], f16, tag="qh")
        ql_sb = q_pool.tile([128, Q], bf16, tag="ql")
        nc.sync.dma_start(out=qh_sb[:], in_=qh[:])
        nc.sync.dma_start(out=ql_sb[:], in_=ql[:])

        for t in range(QTILES):
            ov = out_pool.tile([128, CAND], f32, tag="ov")
            oi = out_pool.tile([128, CAND], u32, tag="oi")
            lh = qh_sb[:, t * 128:(t + 1) * 128]
            ll = ql_sb[:, t * 128:(t + 1) * 128]
            for ci, (off, w) in enumerate(CHUNKS):
                ps = psum_pool.tile([128, CHUNK], f32, tag="ps")
                for j in range(0, w, 512):
                    ww = min(512, w - j)
                    rh = kh_sb[:, off + j: off + j + ww]
                    rl = kl_sb[:, off + j: off + j + ww]
                    po = ps[:, j:j + ww]
                    nc.tensor.matmul(po, lhsT=lh, rhs=rh, start=True, stop=False)
                    nc.tensor.matmul(po, lhsT=lh, rhs=rl, start=False, stop=False)
                    nc.tensor.matmul(po, lhsT=ll, rhs=rh, start=False, stop=True)
                sb = chunk_pool.tile([128, CHUNK], f32, tag="ch")
                nc.scalar.activation(sb[:, :w], ps[:, :w],
                                     mybir.ActivationFunctionType.Copy)
                nc.vector.max(ov[:, ci * 8:(ci + 1) * 8], sb[:, :w])
                nc.vector.max_index(oi[:, ci * 8:(ci + 1) * 8],
                                    ov[:, ci * 8:(ci + 1) * 8], sb[:, :w])
            nc.sync.dma_start(out=outv[t], in_=ov[:])
            nc.sync.dma_start(out=outi[t], in_=oi[:])

    nc.compile()
    return nc


def _get_compiled():
    if "nc" not in _CACHE:
        _CACHE["nc"] = _build_bass()
    return _CACHE["nc"]


def _split_hi_lo(x):
    """x (f32) -> (hi fp16 with subnormals flushed, lo bf16), hi+lo ~ x."""
    import ml_dtypes
    xh = x.astype(np.float16)
    xh = np.where(np.abs(x) < F16_MIN_NORMAL, np.float16(0), xh)
    xl = (x - xh.astype(np.float32)).astype(ml_dtypes.bfloat16)
    return xh, xl


def build_in_maps(queries, K):
    qT = np.ascontiguousarray(queries.reshape(Q, D).T)  # [128, 1024]
    qh, ql = _split_hi_lo(qT)
    in_maps = []
    for c in range(NCORES):
        sh = np.ascontiguousarray(K[c * NSHARD:(c + 1) * NSHARD].T)  # [128, 12500]
        kh, kl = _split_hi_lo(sh)
        in_maps.append({"qh": qh, "ql": ql, "kh": kh, "kl": kl})
    return in_maps


def kernel(queries, K, V, h, e, top_k):
    top_k = int(top_k)
    assert top_k == 32, top_k
    queries = np.asarray(queries, dtype=np.float32)
    K = np.asarray(K, dtype=np.float32)
    V = np.asarray(V, dtype=np.float32)
    h = np.asarray(h, dtype=np.float32)
    e = np.asarray(e, dtype=np.float32)

    # ---- shard + run on 8 NeuronCores ----
    in_maps = build_in_maps(queries, K)

    from concourse.bass_utils import run_bass_kernel_spmd

    nc = _get_compiled()
    res = run_bass_kernel_spmd(nc, in_maps, list(range(NCORES))).results

    # ---- unshard: merge the 8*56 candidates per query ----
    vals = np.stack([res[c]["outv"] for c in range(NCORES)])  # [8, QTILES, 128, CAND]
    idxs = np.stack([res[c]["outi"] for c in range(NCORES)]).astype(np.int64)
    cand_off = np.repeat(np.array([off for off, _ in CHUNKS], dtype=np.int64), 8)
    gidx = idxs + cand_off[None, None, None, :]
    gidx += (np.arange(NCORES, dtype=np.int64) * NSHARD)[:, None, None, None]

    v = vals.transpose(1, 2, 0, 3).reshape(Q, NCORES * CAND)
    gi = gidx.transpose(1, 2, 0, 3).reshape(Q, NCORES * CAND)

    # RBF weights, computed exactly as the reference does (f32 throughout)
    dist_sq = np.float32(2.0) - np.float32(2.0) * v
    rbf = np.exp(-dist_sq / np.float32(2.0 * SIGMA_READ ** 2)).astype(np.float32)

    # global top-32 by rbf, ties broken by lower center index (lax.top_k order)
    order = np.lexsort((gi, -rbf.astype(np.float64)), axis=1)[:, :top_k]
    topk_idx = np.take_along_axis(gi, order, axis=1)  # [Q, 32]
    topk_w = np.take_along_axis(rbf, order, axis=1)  # [Q, 32]

    # ---- final O(k) reduction, replicating the reference numerics ----
    h_topk = h[topk_idx]
    log_w = np.log(topk_w + np.float32(EPS)) + np.log(h_topk + np.float32(EPS))
    m = log_w.max(axis=-1, keepdims=True)
    ew = np.exp(log_w - m)
    weights = (ew / ew.sum(axis=-1, keepdims=True)).astype(np.float32)

    V_sel = V[topk_idx]  # [Q, 32, DV]
    e_sel = e[topk_idx]  # [Q, 32, DE]
    r_V = np.einsum('qk,qkv->qv', weights, V_sel).astype(np.float32)
    r_E = np.einsum('qk,qke->qe', weights, e_sel).astype(np.float32)

    return (
        r_V.reshape(B, T, DV),
        r_E.reshape(B, T, DE),
        weights.reshape(B, T, top_k),
        topk_idx.reshape(B, T, top_k).astype(np.int32),
    )


# revision 8
# speedup vs baseline: 1.0316x; 1.0092x over previous
"""Distributed MemoryCenters read kernel for 8 Trainium2 NeuronCores.

Strategy (sharded-kNN per the distributed top-k pattern):
  - Shard the center table K along n_centers across the 8 cores
    (12500 centers each). Queries are replicated.
  - Each core computes sim = q @ K_shard^T on the PE. To get fp32-grade
    precision at full PE rate, operands are split hi/lo (hi = fp16 with
    subnormals flushed on host, lo = bf16 residual) and accumulated as
    qh*Kh + qh*Kl + ql*Kh in one PSUM group (measured max err 1.9e-7,
    same as fp32).
  - Each core extracts top-8 candidates (values + in-chunk indices) per
    2048-wide chunk of its shard with the DVE max8 / find_index8 ops.
  - The host merges the 8 * 56 = 448 candidates per query, takes the
    global top-32 by RBF weight (reproducing the reference's ordering and
    tie-breaking), and performs the cheap O(k) softmax / gather reduction.

Exactness: top-8 per 2048-chunk covers the global top-32 as long as no
chunk holds more than 8 of the top-32 (actual maximum on this data is 5;
test.py's saturation check proves this per-run).
"""

import numpy as np

SIGMA_READ = 0.5
EPS = 1e-8

B, T, D = 2, 512, 128
N, DV, DE = 100000, 256, 4
NCORES = 8
NSHARD = N // NCORES  # 12500
Q = B * T  # 1024
QTILES = Q // 128  # 8

CHUNK = 2048
# (offset, width) chunks covering the 12500-wide shard
CHUNKS = [(i * CHUNK, min(CHUNK, NSHARD - i * CHUNK)) for i in range((NSHARD + CHUNK - 1) // CHUNK)]
NCHUNK = len(CHUNKS)  # 7
CAND = NCHUNK * 8  # 56 candidates per (query, core)

F16_MIN_NORMAL = 6.2e-05  # flush-to-zero threshold for the hi fp16 part

_CACHE = {}


def _build_bass():
    """Build + compile the per-core Bass program (identical on all cores)."""
    from contextlib import ExitStack

    import concourse.bacc as bacc
    import concourse.mybir as mybir
    import concourse.tile as tile

    f32 = mybir.dt.float32
    f16 = mybir.dt.float16
    bf16 = mybir.dt.bfloat16
    u32 = mybir.dt.uint32

    nc = bacc.Bacc("TRN2", target_bir_lowering=False, debug=False,
                   enable_asserts=False, num_devices=NCORES)

    qh = nc.dram_tensor("qh", [128, Q], f16, kind="ExternalInput").ap()
    ql = nc.dram_tensor("ql", [128, Q], bf16, kind="ExternalInput").ap()
    kh = nc.dram_tensor("kh", [128, NSHARD], f16, kind="ExternalInput").ap()
    kl = nc.dram_tensor("kl", [128, NSHARD], bf16, kind="ExternalInput").ap()
    outv = nc.dram_tensor("outv", [QTILES, 128, CAND], f32, kind="ExternalOutput").ap()
    outi = nc.dram_tensor("outi", [QTILES, 128, CAND], u32, kind="ExternalOutput").ap()

    with tile.TileContext(nc) as tc, ExitStack() as ctx:
        k_pool = ctx.enter_context(tc.tile_pool(name="k", bufs=1))
        q_pool = ctx.enter_context(tc.tile_pool(name="q", bufs=1))
        # 2-bank PSUM pieces, 4 in flight: lets the PE run ahead of the
        # ACT drain so it can ramp to its top p-state
        psum_pool = ctx.enter_context(tc.tile_pool(name="ps", bufs=4, space="PSUM"))
        # deep SBUF runway so ACT (and transitively PE) is not paced by DVE
        chunk_pool = ctx.enter_context(tc.tile_pool(name="ch", bufs=7))
        out_pool = ctx.enter_context(tc.tile_pool(name="out", bufs=2))

        kh_sb = k_pool.tile([128, NSHARD], f16, tag="kh")
        kl_sb = k_pool.tile([128, NSHARD], bf16, tag="kl")
        ndma = 5
        step = (NSHARD + ndma - 1) // ndma
        for i in range(ndma):
            s, e2 = i * step, min(NSHARD, (i + 1) * step)
            nc.sync.dma_start(out=kh_sb[:, s:e2], in_=kh[:, s:e2])
            nc.sync.dma_start(out=kl_sb[:, s:e2], in_=kl[:, s:e2])
        qh_sb = q_pool.tile([128, Q], f16, tag="qh")
        ql_sb = q_pool.tile([128, Q], bf16, tag="ql")
        nc.sync.dma_start(out=qh_sb[:], in_=qh[:])
        nc.sync.dma_start(out=ql_sb[:], in_=ql[:])

        for t in range(QTILES):
            ov = out_pool.tile([128, CAND], f32, tag="ov")
            oi = out_pool.tile([128, CAND], u32, tag="oi")
            lh = qh_sb[:, t * 128:(t + 1) * 128]
            ll = ql_sb[:, t * 128:(t + 1) * 128]
            for ci, (off, w) in enumerate(CHUNKS):
                sb = chunk_pool.tile([128, CHUNK], f32, tag="ch")
                # PSUM pieces of 1024 (2 banks); ACT drains each piece into
                # its slice of the chunk's SBUF buffer
                for p0 in range(0, w, 1024):
                    pw = min(1024, w - p0)
                    ps = psum_pool.tile([128, 1024], f32, tag="ps")
                    for j in range(0, pw, 512):
                        ww = min(512, pw - j)
                        rh = kh_sb[:, off + p0 + j: off + p0 + j + ww]
                        rl = kl_sb[:, off + p0 + j: off + p0 + j + ww]
                        po = ps[:, j:j + ww]
                        nc.tensor.matmul(po, lhsT=lh, rhs=rh, start=True, stop=False)
                        nc.tensor.matmul(po, lhsT=lh, rhs=rl, start=False, stop=False)
                        nc.tensor.matmul(po, lhsT=ll, rhs=rh, start=False, stop=True)
                    nc.scalar.activation(sb[:, p0:p0 + pw], ps[:, :pw],
                                         mybir.ActivationFunctionType.Copy)
                nc.vector.max(ov[:, ci * 8:(ci + 1) * 8], sb[:, :w])
                nc.vector.max_index(oi[:, ci * 8:(ci + 1) * 8],
                                    ov[:, ci * 8:(ci + 1) * 8], sb[:, :w])
            nc.sync.dma_start(out=outv[t], in_=ov[:])
            nc.sync.dma_start(out=outi[t], in_=oi[:])

    nc.compile()
    return nc


def _get_compiled():
    if "nc" not in _CACHE:
        _CACHE["nc"] = _build_bass()
    return _CACHE["nc"]


def _split_hi_lo(x):
    """x (f32) -> (hi fp16 with subnormals flushed, lo bf16), hi+lo ~ x."""
    import ml_dtypes
    xh = x.astype(np.float16)
    xh = np.where(np.abs(x) < F16_MIN_NORMAL, np.float16(0), xh)
    xl = (x - xh.astype(np.float32)).astype(ml_dtypes.bfloat16)
    return xh, xl


def build_in_maps(queries, K):
    qT = np.ascontiguousarray(queries.reshape(Q, D).T)  # [128, 1024]
    qh, ql = _split_hi_lo(qT)
    in_maps = []
    for c in range(NCORES):
        sh = np.ascontiguousarray(K[c * NSHARD:(c + 1) * NSHARD].T)  # [128, 12500]
        kh, kl = _split_hi_lo(sh)
        in_maps.append({"qh": qh, "ql": ql, "kh": kh, "kl": kl})
    return in_maps


def kernel(queries, K, V, h, e, top_k):
    top_k = int(top_k)
    assert top_k == 32, top_k
    queries = np.asarray(queries, dtype=np.float32)
    K = np.asarray(K, dtype=np.float32)
    V = np.asarray(V, dtype=np.float32)
    h = np.asarray(h, dtype=np.float32)
    e = np.asarray(e, dtype=np.float32)

    # ---- shard + run on 8 NeuronCores ----
    in_maps = build_in_maps(queries, K)

    from concourse.bass_utils import run_bass_kernel_spmd

    nc = _get_compiled()
    res = run_bass_kernel_spmd(nc, in_maps, list(range(NCORES))).results

    # ---- unshard: merge the 8*56 candidates per query ----
    vals = np.stack([res[c]["outv"] for c in range(NCORES)])  # [8, QTILES, 128, CAND]
    idxs = np.stack([res[c]["outi"] for c in range(NCORES)]).astype(np.int64)
    cand_off = np.repeat(np.array([off for off, _ in CHUNKS], dtype=np.int64), 8)
    gidx = idxs + cand_off[None, None, None, :]
    gidx += (np.arange(NCORES, dtype=np.int64) * NSHARD)[:, None, None, None]

    v = vals.transpose(1, 2, 0, 3).reshape(Q, NCORES * CAND)
    gi = gidx.transpose(1, 2, 0, 3).reshape(Q, NCORES * CAND)

    # RBF weights, computed exactly as the reference does (f32 throughout)
    dist_sq = np.float32(2.0) - np.float32(2.0) * v
    rbf = np.exp(-dist_sq / np.float32(2.0 * SIGMA_READ ** 2)).astype(np.float32)

    # global top-32 by rbf, ties broken by lower center index (lax.top_k order)
    order = np.lexsort((gi, -rbf.astype(np.float64)), axis=1)[:, :top_k]
    topk_idx = np.take_along_axis(gi, order, axis=1)  # [Q, 32]
    topk_w = np.take_along_axis(rbf, order, axis=1)  # [Q, 32]

    # ---- final O(k) reduction, replicating the reference numerics ----
    h_topk = h[topk_idx]
    log_w = np.log(topk_w + np.float32(EPS)) + np.log(h_topk + np.float32(EPS))
    m = log_w.max(axis=-1, keepdims=True)
    ew = np.exp(log_w - m)
    weights = (ew / ew.sum(axis=-1, keepdims=True)).astype(np.float32)

    V_sel = V[topk_idx]  # [Q, 32, DV]
    e_sel = e[topk_idx]  # [Q, 32, DE]
    r_V = np.einsum('qk,qkv->qv', weights, V_sel).astype(np.float32)
    r_E = np.einsum('qk,qke->qe', weights, e_sel).astype(np.float32)

    return (
        r_V.reshape(B, T, DV),
        r_E.reshape(B, T, DE),
        weights.reshape(B, T, top_k),
        topk_idx.reshape(B, T, top_k).astype(np.int32),
    )


# revision 10
# speedup vs baseline: 1.0652x; 1.0326x over previous
"""Distributed MemoryCenters read kernel for 8 Trainium2 NeuronCores.

Strategy (sharded-kNN per the distributed top-k pattern):
  - Shard the center table K along n_centers across the 8 cores
    (12500 centers each). Queries are replicated.
  - Each core computes sim = q @ K_shard^T on the PE. To get fp32-grade
    precision at full PE rate, operands are split hi/lo (hi = fp16 with
    subnormals flushed on host, lo = bf16 residual) and accumulated as
    qh*Kh + qh*Kl + ql*Kh in one PSUM group (measured max err 1.9e-7,
    same as fp32).
  - Each core extracts top-8 candidates (values + in-chunk indices) per
    2048-wide chunk of its shard with the DVE max8 / find_index8 ops.
  - The host merges the 8 * 56 = 448 candidates per query, takes the
    global top-32 by RBF weight (reproducing the reference's ordering and
    tie-breaking), and performs the cheap O(k) softmax / gather reduction.

Exactness: top-8 per 2048-chunk covers the global top-32 as long as no
chunk holds more than 8 of the top-32 (actual maximum on this data is 5;
test.py's saturation check proves this per-run).
"""

import numpy as np

SIGMA_READ = 0.5
EPS = 1e-8

B, T, D = 2, 512, 128
N, DV, DE = 100000, 256, 4
NCORES = 8
NSHARD = N // NCORES  # 12500
Q = B * T  # 1024
QTILES = Q // 128  # 8

CHUNK = 2048
# (offset, width) chunks covering the 12500-wide shard
CHUNKS = [(i * CHUNK, min(CHUNK, NSHARD - i * CHUNK)) for i in range((NSHARD + CHUNK - 1) // CHUNK)]
NCHUNK = len(CHUNKS)  # 7
CAND = NCHUNK * 8  # 56 candidates per (query, core)

F16_MIN_NORMAL = 6.2e-05  # flush-to-zero threshold for the hi fp16 part

_CACHE = {}


def _build_bass():
    """Build + compile the per-core Bass program (identical on all cores)."""
    from contextlib import ExitStack

    import concourse.bacc as bacc
    import concourse.mybir as mybir
    import concourse.tile as tile

    f32 = mybir.dt.float32
    f16 = mybir.dt.float16
    bf16 = mybir.dt.bfloat16
    u32 = mybir.dt.uint32

    nc = bacc.Bacc("TRN2", target_bir_lowering=False, debug=False,
                   enable_asserts=False, num_devices=NCORES)

    qh = nc.dram_tensor("qh", [128, Q], f16, kind="ExternalInput").ap()
    ql = nc.dram_tensor("ql", [128, Q], bf16, kind="ExternalInput").ap()
    kh = nc.dram_tensor("kh", [128, NSHARD], f16, kind="ExternalInput").ap()
    kl = nc.dram_tensor("kl", [128, NSHARD], bf16, kind="ExternalInput").ap()
    outv = nc.dram_tensor("outv", [QTILES, 128, CAND], f32, kind="ExternalOutput").ap()
    outi = nc.dram_tensor("outi", [QTILES, 128, CAND], u32, kind="ExternalOutput").ap()

    with tile.TileContext(nc) as tc, ExitStack() as ctx:
        k_pool = ctx.enter_context(tc.tile_pool(name="k", bufs=1))
        q_pool = ctx.enter_context(tc.tile_pool(name="q", bufs=1))
        # 2-bank PSUM pieces, 4 in flight: lets the PE run ahead of the
        # ACT drain so it can ramp to its top p-state
        psum_pool = ctx.enter_context(tc.tile_pool(name="ps", bufs=4, space="PSUM"))
        # deep SBUF runway so ACT (and transitively PE) is not paced by DVE
        chunk_pool = ctx.enter_context(tc.tile_pool(name="ch", bufs=7))
        out_pool = ctx.enter_context(tc.tile_pool(name="out", bufs=2))

        # queries first (the PE's ldweights gate on them), on both HWDGE rings
        qh_sb = q_pool.tile([128, Q], f16, tag="qh")
        ql_sb = q_pool.tile([128, Q], bf16, tag="ql")
        nc.sync.dma_start(out=qh_sb[:], in_=qh[:])
        nc.scalar.dma_start(out=ql_sb[:], in_=ql[:])

        # K in chunk-pair pieces as separate tiles so early matmuls only
        # gate on the piece they read; kh on the SP ring, kl on the ACT ring
        PIECE = 2 * CHUNK  # 4096
        pieces = [(i * PIECE, min(PIECE, NSHARD - i * PIECE))
                  for i in range((NSHARD + PIECE - 1) // PIECE)]
        kh_sbs, kl_sbs = [], []
        for pi, (s, w) in enumerate(pieces):
            kh_t = k_pool.tile([128, w], f16, tag=f"kh{pi}")
            kl_t = k_pool.tile([128, w], bf16, tag=f"kl{pi}")
            nc.sync.dma_start(out=kh_t[:], in_=kh[:, s:s + w])
            nc.scalar.dma_start(out=kl_t[:], in_=kl[:, s:s + w])
            kh_sbs.append(kh_t)
            kl_sbs.append(kl_t)

        def k_slice(tiles, off, width):
            pi, rel = off // PIECE, off % PIECE
            return tiles[pi][:, rel:rel + width]

        for t in range(QTILES):
            ov = out_pool.tile([128, CAND], f32, tag="ov")
            oi = out_pool.tile([128, CAND], u32, tag="oi")
            lh = qh_sb[:, t * 128:(t + 1) * 128]
            ll = ql_sb[:, t * 128:(t + 1) * 128]
            for ci, (off, w) in enumerate(CHUNKS):
                sb = chunk_pool.tile([128, CHUNK], f32, tag="ch")
                # PSUM pieces of 1024 (2 banks); ACT drains each piece into
                # its slice of the chunk's SBUF buffer
                for p0 in range(0, w, 1024):
                    pw = min(1024, w - p0)
                    ps = psum_pool.tile([128, 1024], f32, tag="ps")
                    for j in range(0, pw, 512):
                        ww = min(512, pw - j)
                        rh = k_slice(kh_sbs, off + p0 + j, ww)
                        rl = k_slice(kl_sbs, off + p0 + j, ww)
                        po = ps[:, j:j + ww]
                        nc.tensor.matmul(po, lhsT=lh, rhs=rh, start=True, stop=False)
                        nc.tensor.matmul(po, lhsT=lh, rhs=rl, start=False, stop=False)
                        nc.tensor.matmul(po, lhsT=ll, rhs=rh, start=False, stop=True)
                    nc.scalar.activation(sb[:, p0:p0 + pw], ps[:, :pw],
                                         mybir.ActivationFunctionType.Copy)
                nc.vector.max(ov[:, ci * 8:(ci + 1) * 8], sb[:, :w])
                nc.vector.max_index(oi[:, ci * 8:(ci + 1) * 8],
                                    ov[:, ci * 8:(ci + 1) * 8], sb[:, :w])
            nc.sync.dma_start(out=outv[t], in_=ov[:])
            nc.sync.dma_start(out=outi[t], in_=oi[:])

    nc.compile()
    return nc


def _get_compiled():
    if "nc" not in _CACHE:
        _CACHE["nc"] = _build_bass()
    return _CACHE["nc"]


def _split_hi_lo(x):
    """x (f32) -> (hi fp16 with subnormals flushed, lo bf16), hi+lo ~ x."""
    import ml_dtypes
    xh = x.astype(np.float16)
    xh = np.where(np.abs(x) < F16_MIN_NORMAL, np.float16(0), xh)
    xl = (x - xh.astype(np.float32)).astype(ml_dtypes.bfloat16)
    return xh, xl


def build_in_maps(queries, K):
    qT = np.ascontiguousarray(queries.reshape(Q, D).T)  # [128, 1024]
    qh, ql = _split_hi_lo(qT)
    in_maps = []
    for c in range(NCORES):
        sh = np.ascontiguousarray(K[c * NSHARD:(c + 1) * NSHARD].T)  # [128, 12500]
        kh, kl = _split_hi_lo(sh)
        in_maps.append({"qh": qh, "ql": ql, "kh": kh, "kl": kl})
    return in_maps


def kernel(queries, K, V, h, e, top_k):
    top_k = int(top_k)
    assert top_k == 32, top_k
    queries = np.asarray(queries, dtype=np.float32)
    K = np.asarray(K, dtype=np.float32)
    V = np.asarray(V, dtype=np.float32)
    h = np.asarray(h, dtype=np.float32)
    e = np.asarray(e, dtype=np.float32)

    # ---- shard + run on 8 NeuronCores ----
    in_maps = build_in_maps(queries, K)

    from concourse.bass_utils import run_bass_kernel_spmd

    nc = _get_compiled()
    res = run_bass_kernel_spmd(nc, in_maps, list(range(NCORES))).results

    # ---- unshard: merge the 8*56 candidates per query ----
    vals = np.stack([res[c]["outv"] for c in range(NCORES)])  # [8, QTILES, 128, CAND]
    idxs = np.stack([res[c]["outi"] for c in range(NCORES)]).astype(np.int64)
    cand_off = np.repeat(np.array([off for off, _ in CHUNKS], dtype=np.int64), 8)
    gidx = idxs + cand_off[None, None, None, :]
    gidx += (np.arange(NCORES, dtype=np.int64) * NSHARD)[:, None, None, None]

    v = vals.transpose(1, 2, 0, 3).reshape(Q, NCORES * CAND)
    gi = gidx.transpose(1, 2, 0, 3).reshape(Q, NCORES * CAND)

    # RBF weights, computed exactly as the reference does (f32 throughout)
    dist_sq = np.float32(2.0) - np.float32(2.0) * v
    rbf = np.exp(-dist_sq / np.float32(2.0 * SIGMA_READ ** 2)).astype(np.float32)

    # global top-32 by rbf, ties broken by lower center index (lax.top_k order)
    order = np.lexsort((gi, -rbf.astype(np.float64)), axis=1)[:, :top_k]
    topk_idx = np.take_along_axis(gi, order, axis=1)  # [Q, 32]
    topk_w = np.take_along_axis(rbf, order, axis=1)  # [Q, 32]

    # ---- final O(k) reduction, replicating the reference numerics ----
    h_topk = h[topk_idx]
    log_w = np.log(topk_w + np.float32(EPS)) + np.log(h_topk + np.float32(EPS))
    m = log_w.max(axis=-1, keepdims=True)
    ew = np.exp(log_w - m)
    weights = (ew / ew.sum(axis=-1, keepdims=True)).astype(np.float32)

    V_sel = V[topk_idx]  # [Q, 32, DV]
    e_sel = e[topk_idx]  # [Q, 32, DE]
    r_V = np.einsum('qk,qkv->qv', weights, V_sel).astype(np.float32)
    r_E = np.einsum('qk,qke->qe', weights, e_sel).astype(np.float32)

    return (
        r_V.reshape(B, T, DV),
        r_E.reshape(B, T, DE),
        weights.reshape(B, T, top_k),
        topk_idx.reshape(B, T, top_k).astype(np.int32),
    )


# revision 15
# speedup vs baseline: 1.0784x; 1.0124x over previous
"""Distributed MemoryCenters read kernel for 8 Trainium2 NeuronCores.

Strategy (sharded-kNN per the distributed top-k pattern):
  - Shard the center table K along n_centers across the 8 cores
    (12500 centers each). Queries are replicated.
  - Each core computes sim = q @ K_shard^T on the PE. To get fp32-grade
    precision at full PE rate, operands are split hi/lo (hi = fp16 with
    subnormals flushed on host, lo = bf16 residual) and accumulated as
    qh*Kh + qh*Kl + ql*Kh in one PSUM group (measured max err 1.9e-7,
    same as fp32).
  - Each core extracts top-8 candidates (values + in-chunk indices) per
    2048-wide chunk of its shard with the DVE max8 / find_index8 ops.
  - The host merges the 8 * 56 = 448 candidates per query, takes the
    global top-32 by RBF weight (reproducing the reference's ordering and
    tie-breaking), and performs the cheap O(k) softmax / gather reduction.

Exactness: top-8 per 2048-chunk covers the global top-32 as long as no
chunk holds more than 8 of the top-32 (actual maximum on this data is 5;
test.py's saturation check proves this per-run).
"""

import numpy as np

SIGMA_READ = 0.5
EPS = 1e-8

B, T, D = 2, 512, 128
N, DV, DE = 100000, 256, 4
NCORES = 8
NSHARD = N // NCORES  # 12500
Q = B * T  # 1024
QTILES = Q // 128  # 8

CHUNK = 2048
# chunk widths covering the 12500-wide shard; small leading chunks let the
# DVE start as soon as the first matmul pieces land (pipeline fill)
_WIDTHS = [512, 512, 1024, 2048, 2048, 2048, 2048, 2260]
assert sum(_WIDTHS) == NSHARD
CHUNKS = []
_off = 0
for _w in _WIDTHS:
    CHUNKS.append((_off, _w))
    _off += _w
NCHUNK = len(CHUNKS)  # 9
CAND = NCHUNK * 8  # 72 candidates per (query, core)

F16_MIN_NORMAL = 6.2e-05  # flush-to-zero threshold for the hi fp16 part

_CACHE = {}


def _build_bass():
    """Build + compile the per-core Bass program (identical on all cores)."""
    from contextlib import ExitStack

    import concourse.bacc as bacc
    import concourse.mybir as mybir
    import concourse.tile as tile

    f32 = mybir.dt.float32
    f16 = mybir.dt.float16
    bf16 = mybir.dt.bfloat16
    u32 = mybir.dt.uint32

    nc = bacc.Bacc("TRN2", target_bir_lowering=False, debug=False,
                   enable_asserts=False, num_devices=NCORES)

    qh = nc.dram_tensor("qh", [128, Q], f16, kind="ExternalInput").ap()
    ql = nc.dram_tensor("ql", [128, Q], bf16, kind="ExternalInput").ap()
    kh = nc.dram_tensor("kh", [128, NSHARD], f16, kind="ExternalInput").ap()
    kl = nc.dram_tensor("kl", [128, NSHARD], bf16, kind="ExternalInput").ap()
    outv = nc.dram_tensor("outv", [QTILES, 128, CAND], f32, kind="ExternalOutput").ap()
    outi = nc.dram_tensor("outi", [QTILES, 128, CAND], u32, kind="ExternalOutput").ap()

    with tile.TileContext(nc) as tc, ExitStack() as ctx:
        k_pool = ctx.enter_context(tc.tile_pool(name="k", bufs=1))
        q_pool = ctx.enter_context(tc.tile_pool(name="q", bufs=1))
        # 2-bank PSUM pieces, 4 in flight: lets the PE run ahead of the
        # ACT drain so it can ramp to its top p-state
        psum_pool = ctx.enter_context(tc.tile_pool(name="ps", bufs=4, space="PSUM"))
        # deep SBUF runway so ACT (and transitively PE) is not paced by DVE
        chunk_pool = ctx.enter_context(tc.tile_pool(name="ch", bufs=7))
        out_pool = ctx.enter_context(tc.tile_pool(name="out", bufs=2))

        # queries first (the PE's ldweights gate on them), on both HWDGE rings
        qh_sb = q_pool.tile([128, Q], f16, tag="qh")
        ql_sb = q_pool.tile([128, Q], bf16, tag="ql")
        nc.sync.dma_start(out=qh_sb[:], in_=qh[:])
        nc.scalar.dma_start(out=ql_sb[:], in_=ql[:])

        # K in chunk-aligned pieces as separate tiles so early matmuls only
        # gate on the piece they read; kh on the SP ring, kl on the ACT ring
        piece_bounds = [0, 512, 2048, 6144, 10240, NSHARD]
        pieces = [(piece_bounds[i], piece_bounds[i + 1] - piece_bounds[i])
                  for i in range(len(piece_bounds) - 1)]
        kh_sbs, kl_sbs = [], []
        for pi, (s, w) in enumerate(pieces):
            kh_t = k_pool.tile([128, w], f16, tag=f"kh{pi}")
            kl_t = k_pool.tile([128, w], bf16, tag=f"kl{pi}")
            nc.sync.dma_start(out=kh_t[:], in_=kh[:, s:s + w])
            nc.scalar.dma_start(out=kl_t[:], in_=kl[:, s:s + w])
            kh_sbs.append(kh_t)
            kl_sbs.append(kl_t)

        def k_slice(tiles, off, width):
            for (s, w), t in zip(pieces, tiles):
                if s <= off and off + width <= s + w:
                    return t[:, off - s:off - s + width]
            raise AssertionError((off, width))

        for t in range(QTILES):
            ov = out_pool.tile([128, CAND], f32, tag="ov")
            oi = out_pool.tile([128, CAND], u32, tag="oi")
            lh = qh_sb[:, t * 128:(t + 1) * 128]
            ll = ql_sb[:, t * 128:(t + 1) * 128]
            for ci, (off, w) in enumerate(CHUNKS):
                sb = chunk_pool.tile([128, max(_WIDTHS)], f32, tag="ch")
                # PSUM pieces of 1024 (2 banks); ACT drains each piece into
                # its slice of the chunk's SBUF buffer
                for p0 in range(0, w, 1024):
                    pw = min(1024, w - p0)
                    ps = psum_pool.tile([128, 1024], f32, tag="ps")
                    for j in range(0, pw, 512):
                        ww = min(512, pw - j)
                        rh = k_slice(kh_sbs, off + p0 + j, ww)
                        rl = k_slice(kl_sbs, off + p0 + j, ww)
                        po = ps[:, j:j + ww]
                        nc.tensor.matmul(po, lhsT=lh, rhs=rh, start=True, stop=False)
                        nc.tensor.matmul(po, lhsT=lh, rhs=rl, start=False, stop=False)
                        nc.tensor.matmul(po, lhsT=ll, rhs=rh, start=False, stop=True)
                    nc.scalar.activation(sb[:, p0:p0 + pw], ps[:, :pw],
                                         mybir.ActivationFunctionType.Copy)
                nc.vector.max(ov[:, ci * 8:(ci + 1) * 8], sb[:, :w])
                nc.vector.max_index(oi[:, ci * 8:(ci + 1) * 8],
                                    ov[:, ci * 8:(ci + 1) * 8], sb[:, :w])
            nc.sync.dma_start(out=outv[t], in_=ov[:])
            nc.sync.dma_start(out=outi[t], in_=oi[:])

    nc.compile()
    return nc


def _get_compiled():
    if "nc" not in _CACHE:
        _CACHE["nc"] = _build_bass()
    return _CACHE["nc"]


def _split_hi_lo(x):
    """x (f32) -> (hi fp16 with subnormals flushed, lo bf16), hi+lo ~ x."""
    import ml_dtypes
    xh = x.astype(np.float16)
    xh = np.where(np.abs(x) < F16_MIN_NORMAL, np.float16(0), xh)
    xl = (x - xh.astype(np.float32)).astype(ml_dtypes.bfloat16)
    return xh, xl


def build_in_maps(queries, K):
    qT = np.ascontiguousarray(queries.reshape(Q, D).T)  # [128, 1024]
    qh, ql = _split_hi_lo(qT)
    in_maps = []
    for c in range(NCORES):
        sh = np.ascontiguousarray(K[c * NSHARD:(c + 1) * NSHARD].T)  # [128, 12500]
        kh, kl = _split_hi_lo(sh)
        in_maps.append({"qh": qh, "ql": ql, "kh": kh, "kl": kl})
    return in_maps


def kernel(queries, K, V, h, e, top_k):
    top_k = int(top_k)
    assert top_k == 32, top_k
    queries = np.asarray(queries, dtype=np.float32)
    K = np.asarray(K, dtype=np.float32)
    V = np.asarray(V, dtype=np.float32)
    h = np.asarray(h, dtype=np.float32)
    e = np.asarray(e, dtype=np.float32)

    # ---- shard + run on 8 NeuronCores ----
    in_maps = build_in_maps(queries, K)

    from concourse.bass_utils import run_bass_kernel_spmd

    nc = _get_compiled()
    res = run_bass_kernel_spmd(nc, in_maps, list(range(NCORES))).results

    # ---- unshard: merge the 8*56 candidates per query ----
    vals = np.stack([res[c]["outv"] for c in range(NCORES)])  # [8, QTILES, 128, CAND]
    idxs = np.stack([res[c]["outi"] for c in range(NCORES)]).astype(np.int64)
    cand_off = np.repeat(np.array([off for off, _ in CHUNKS], dtype=np.int64), 8)
    gidx = idxs + cand_off[None, None, None, :]
    gidx += (np.arange(NCORES, dtype=np.int64) * NSHARD)[:, None, None, None]

    v = vals.transpose(1, 2, 0, 3).reshape(Q, NCORES * CAND)
    gi = gidx.transpose(1, 2, 0, 3).reshape(Q, NCORES * CAND)

    # RBF weights, computed exactly as the reference does (f32 throughout)
    dist_sq = np.float32(2.0) - np.float32(2.0) * v
    rbf = np.exp(-dist_sq / np.float32(2.0 * SIGMA_READ ** 2)).astype(np.float32)

    # global top-32 by rbf, ties broken by lower center index (lax.top_k order)
    order = np.lexsort((gi, -rbf.astype(np.float64)), axis=1)[:, :top_k]
    topk_idx = np.take_along_axis(gi, order, axis=1)  # [Q, 32]
    topk_w = np.take_along_axis(rbf, order, axis=1)  # [Q, 32]

    # ---- final O(k) reduction, replicating the reference numerics ----
    h_topk = h[topk_idx]
    log_w = np.log(topk_w + np.float32(EPS)) + np.log(h_topk + np.float32(EPS))
    m = log_w.max(axis=-1, keepdims=True)
    ew = np.exp(log_w - m)
    weights = (ew / ew.sum(axis=-1, keepdims=True)).astype(np.float32)

    V_sel = V[topk_idx]  # [Q, 32, DV]
    e_sel = e[topk_idx]  # [Q, 32, DE]
    r_V = np.einsum('qk,qkv->qv', weights, V_sel).astype(np.float32)
    r_E = np.einsum('qk,qke->qe', weights, e_sel).astype(np.float32)

    return (
        r_V.reshape(B, T, DV),
        r_E.reshape(B, T, DE),
        weights.reshape(B, T, top_k),
        topk_idx.reshape(B, T, top_k).astype(np.int32),
    )
